# revision 1
# baseline (speedup 1.0000x reference)
"""MAMDense kernel for Trainium2 (8 NeuronCores, SPMD over row shards).

C[i,j] = max_k(x[i,k]*W[j,k]) + min_k(x[i,k]*W[j,k]) + bias[j]

Strategy (fp32, bit-exact vs the fp32 reference):
  - Shard the flattened row dim M=2048 across 8 cores (256 rows each).
  - Per core, layout A: partitions = rows (2 tiles of 128), free dim = N,
    iterate k. Per k, one broadcast tile Bcast_k = W^T[k,:] replicated to
    128 partitions (0-stride DMA from HBM), then fused multiply+compare
    on the Vector engine via scalar_tensor_tensor:
        acc_max = max(Bcast_k * x[:,k], acc_max)   (in-place)
        acc_min = min(Bcast_k * x[:,k], acc_min)
  - Final: C = acc_max + acc_min + bias_bcast, DMA out (natural layout).

This walrus build only accepts ONE semaphore wait per instruction, so a
post-pass splits extra waits onto preceding NoOps, and the Tile tail
drain is patched the same way.
"""

import os
import numpy as np

M_FULL, K, N, NCORES = 2048, 768, 768, 8
MC = M_FULL // NCORES
FMAX = float(np.finfo(np.float32).max)

_STATE = {}
LAST_RUN_SECONDS = None


def _patch_tile_drain(tile, mybir, ScopedClock, maxw=1):
    if getattr(tile.TileContext, "_mam_drain_patched", False):
        return

    def _pd(self, tick_clock, wait_clock):
        nc = self.nc
        drain_inst = nc.sync.drain()
        wait_clock.add_sem_waits(
            drain_inst.ins, ScopedClock({None: tick_clock.global_clock})
        )
        si = drain_inst.ins.sync_info
        waits = list(si.on_wait) if si is not None else []
        if len(waits) > maxw:
            si.on_wait = waits[:maxw]
            for i in range(maxw, len(waits), maxw):
                nop = nc.sync.nop(nofuse=True, hint="waitsplit")
                nop.ins.sync_info = mybir.SyncInfo(
                    on_wait=list(waits[i : i + maxw]), on_update=[]
                )
        nc.all_engine_barrier()
        popped = nc._tile_sem_poison_stack.pop()
        assert popped is self._sem_poison
        nc.clear_and_free_semaphores(list(self.sems.allocated().values()))
        nc.all_engine_barrier()

    tile.TileContext._drain_and_barrier = _pd
    tile.TileContext._mam_drain_patched = True


def _split_sem_waits(nc, mybir, maxw=1):
    """Walrus here rejects >1 sync wait per instruction; hoist extras onto
    preceding same-engine NoOps."""
    n = 0
    for f in nc.m.functions:
        for blk in f.blocks:
            insts = blk.instructions
            i = 0
            while i < len(insts):
                inst = insts[i]
                si = inst.sync_info
                if si is not None and len(si.on_wait) > maxw:
                    waits = list(si.on_wait)
                    si.on_wait = waits[:maxw]
                    rest = waits[maxw:]
                    for j in range(0, len(rest), maxw):
                        n += 1
                        nop = mybir.InstNoOp(
                            name=f"I-wsplit-{n}-{inst.name}",
                            engine=inst.engine,
                            ins=[],
                            outs=[],
                            sync_info=mybir.SyncInfo(
                                on_wait=list(rest[j : j + maxw]), on_update=[]
                            ),
                        )
                        nc.register_instruction(nop)
                        insts.insert(i, nop)
                        i += 1
                i += 1
    return n


def _build_nc():
    import concourse.bass as bass
    import concourse.tile as tile
    import concourse.mybir as mybir
    from concourse.vector_clock import ScopedClock

    _patch_tile_drain(tile, mybir, ScopedClock)

    DT = mybir.dt.float32
    nc = bass.Bass("TRN2", debug=False)
    xs = nc.dram_tensor("xs", [MC, K], DT, kind="ExternalInput")
    wt = nc.dram_tensor("wt", [K, N], DT, kind="ExternalInput")  # weight.T
    bias = nc.dram_tensor("bias_in", [N], DT, kind="ExternalInput")
    out = nc.dram_tensor("out", [MC, N], DT, kind="ExternalOutput")
    with tile.TileContext(nc) as tc:
        with tc.tile_pool(name="singles", bufs=1) as singles, tc.tile_pool(
            name="bpool", bufs=8
        ) as bpool:
            x_re = xs.ap().rearrange("(t p) k -> t p k", p=128)
            o_re = out.ap().rearrange("(t p) n -> t p n", p=128)
            ntiles = MC // 128
            x_sb, amax, amin = [], [], []
            for t in range(ntiles):
                xt = singles.tile([128, K], DT, tag=f"x{t}")
                nc.sync.dma_start(out=xt[:], in_=x_re[t])
                x_sb.append(xt)
                mx = singles.tile([128, N], DT, tag=f"amax{t}")
                mn = singles.tile([128, N], DT, tag=f"amin{t}")
                nc.vector.memset(mx[:], -FMAX)
                nc.vector.memset(mn[:], FMAX)
                amax.append(mx)
                amin.append(mn)
            biasb = singles.tile([128, N], DT, tag="biasb")
            bap = bias.ap()
            nc.sync.dma_start(
                out=biasb[:],
                in_=bass.AP(
                    tensor=bap.tensor, offset=bap.offset, ap=[[0, 128], list(bap.ap[0])]
                ),
            )
            for k in range(K):
                bt = bpool.tile([128, N], DT, tag="b")
                row = wt.ap()[k : k + 1, :]
                nc.sync.dma_start(
                    out=bt[:],
                    in_=bass.AP(
                        tensor=row.tensor,
                        offset=row.offset,
                        ap=[[0, 128], list(row.ap[1])],
                    ),
                )
                for t in range(ntiles):
                    nc.vector.scalar_tensor_tensor(
                        out=amax[t][:],
                        in0=bt[:],
                        scalar=x_sb[t][:, k : k + 1],
                        in1=amax[t][:],
                        op0=mybir.AluOpType.mult,
                        op1=mybir.AluOpType.max,
                    )
                    nc.vector.scalar_tensor_tensor(
                        out=amin[t][:],
                        in0=bt[:],
                        scalar=x_sb[t][:, k : k + 1],
                        in1=amin[t][:],
                        op0=mybir.AluOpType.mult,
                        op1=mybir.AluOpType.min,
                    )
            for t in range(ntiles):
                nc.vector.tensor_tensor(
                    out=amax[t][:],
                    in0=amax[t][:],
                    in1=amin[t][:],
                    op=mybir.AluOpType.add,
                )
                nc.vector.tensor_tensor(
                    out=amax[t][:],
                    in0=amax[t][:],
                    in1=biasb[:],
                    op=mybir.AluOpType.add,
                )
                nc.sync.dma_start(out=o_re[t], in_=amax[t][:])
    _split_sem_waits(nc, mybir)
    return nc


def _make_runner(nc, n_cores=NCORES):
    import jax
    from jax.sharding import Mesh, PartitionSpec
    from jax.experimental.shard_map import shard_map
    import concourse.mybir as mybir
    from concourse import bass2jax

    bass2jax.install_neuronx_cc_hook()

    partition_name = (
        nc.partition_id_tensor.name if nc.partition_id_tensor else None
    )
    in_names, out_names, out_avals, zero_shapes = [], [], [], []
    for alloc in nc.m.functions[0].allocations:
        if not isinstance(alloc, mybir.MemoryLocationSet):
            continue
        name = alloc.memorylocations[0].name
        if alloc.kind == "ExternalInput":
            if name != partition_name:
                in_names.append(name)
        elif alloc.kind == "ExternalOutput":
            shape = tuple(alloc.tensor_shape)
            dtype = mybir.dt.np(alloc.dtype)
            out_names.append(name)
            out_avals.append(jax.core.ShapedArray(shape, dtype))
            zero_shapes.append((shape, dtype))
    n_params = len(in_names)
    n_outs = len(out_avals)
    in_names_all = list(in_names) + list(out_names)
    if partition_name is not None:
        in_names_all.append(partition_name)

    def _body(*args):
        operands = list(args)
        if partition_name is not None:
            operands.append(bass2jax.partition_id_tensor())
        outs = bass2jax._bass_exec_p.bind(
            *operands,
            out_avals=tuple(out_avals),
            in_names=tuple(in_names_all),
            out_names=tuple(out_names),
            lowering_input_output_aliases=(),
            sim_require_finite=True,
            sim_require_nnan=True,
            nc=nc,
        )
        return tuple(outs)

    devices = jax.devices()[:n_cores]
    mesh = Mesh(np.asarray(devices), ("core",))
    in_specs = (PartitionSpec("core"),) * (n_params + n_outs)
    out_specs = (PartitionSpec("core"),) * n_outs
    sharded = jax.jit(
        shard_map(
            _body, mesh=mesh, in_specs=in_specs, out_specs=out_specs, check_rep=False
        ),
        keep_unused=True,
    )

    def run(in_maps):
        global LAST_RUN_SECONDS
        import time as _time

        per_core = [[np.asarray(m[nm]) for nm in in_names] for m in in_maps]
        concat_in = [
            np.concatenate([per_core[c][i] for c in range(n_cores)], axis=0)
            for i in range(n_params)
        ]
        concat_zeros = [
            np.zeros((n_cores * s[0], *s[1:]), d) for (s, d) in zero_shapes
        ]
        t0 = _time.time()
        out_arrs = sharded(*concat_in, *concat_zeros)
        out_np = [np.asarray(a) for a in out_arrs]
        LAST_RUN_SECONDS = _time.time() - t0
        return [
            {
                nm: out_np[i].reshape(n_cores, *out_avals[i].shape)[c]
                for i, nm in enumerate(out_names)
            }
            for c in range(n_cores)
        ]

    run.sharded = sharded
    run.in_names = in_names
    run.zero_shapes = zero_shapes
    run.out_names = out_names
    run.out_avals = out_avals
    run.mesh = mesh
    return run


def _get_runner():
    if "runner" not in _STATE:
        nc = _build_nc()
        try:
            _STATE["runner"] = _make_runner(nc)
        except Exception:
            # Fallback: plain (re-jitting) path through bass_utils.
            from concourse.bass_utils import run_bass_kernel_spmd

            def run(in_maps):
                res = run_bass_kernel_spmd(
                    nc, in_maps, core_ids=list(range(NCORES))
                )
                return res.results

            _STATE["runner"] = run
    return _STATE["runner"]


def kernel(x, weight, bias):
    x = np.ascontiguousarray(np.asarray(x, dtype=np.float32))
    W = np.ascontiguousarray(np.asarray(weight, dtype=np.float32))
    b = np.ascontiguousarray(np.asarray(bias, dtype=np.float32))
    run = _get_runner()
    xf = x.reshape(-1, K)
    wt = np.ascontiguousarray(W.T)
    in_maps = [
        {"xs": xf[c * MC : (c + 1) * MC], "wt": wt, "bias_in": b}
        for c in range(NCORES)
    ]
    outs = run(in_maps)
    C = np.concatenate([o["out"] for o in outs], axis=0)
    return np.ascontiguousarray(C.reshape(x.shape[:-1] + (W.shape[0],)), dtype=np.float32)


# revision 13
# speedup vs baseline: 175.6066x; 175.6066x over previous
"""MAMDense kernel for Trainium2 (8 NeuronCores, SPMD over row shards).

C[i,j] = max_k(x[i,k]*W[j,k]) + min_k(x[i,k]*W[j,k]) + bias[j]

Strategy (fp32, bit-exact vs the fp32 reference):
  - Shard the flattened row dim M=2048 across 8 cores (256 rows each).
  - Per core, layout A: partitions = rows (2 tiles of 128), free dim = N,
    iterate k. Per k, one broadcast tile Bcast_k = W^T[k,:] replicated to
    128 partitions (0-stride DMA from HBM), then fused multiply+compare
    on the Vector engine via scalar_tensor_tensor:
        acc_max = max(Bcast_k * x[:,k], acc_max)   (in-place)
        acc_min = min(Bcast_k * x[:,k], acc_min)
  - Final: C = acc_max + acc_min + bias_bcast, DMA out (natural layout).

This walrus build only accepts ONE semaphore wait per instruction, so a
post-pass splits extra waits onto preceding NoOps, and the Tile tail
drain is patched the same way.
"""

import os
import numpy as np

M_FULL, K, N, NCORES = 2048, 768, 768, 8
MC = M_FULL // NCORES
FMAX = float(np.finfo(np.float32).max)

_STATE = {}
LAST_RUN_SECONDS = None


def _patch_tile_drain(tile, mybir, ScopedClock, maxw=1):
    if getattr(tile.TileContext, "_mam_drain_patched", False):
        return

    def _pd(self, tick_clock, wait_clock):
        nc = self.nc
        drain_inst = nc.sync.drain()
        wait_clock.add_sem_waits(
            drain_inst.ins, ScopedClock({None: tick_clock.global_clock})
        )
        si = drain_inst.ins.sync_info
        waits = list(si.on_wait) if si is not None else []
        if len(waits) > maxw:
            si.on_wait = waits[:maxw]
            for i in range(maxw, len(waits), maxw):
                nop = nc.sync.nop(nofuse=True, hint="waitsplit")
                nop.ins.sync_info = mybir.SyncInfo(
                    on_wait=list(waits[i : i + maxw]), on_update=[]
                )
        nc.all_engine_barrier()
        popped = nc._tile_sem_poison_stack.pop()
        assert popped is self._sem_poison
        nc.clear_and_free_semaphores(list(self.sems.allocated().values()))
        nc.all_engine_barrier()

    tile.TileContext._drain_and_barrier = _pd
    tile.TileContext._mam_drain_patched = True


def _split_sem_waits(nc, mybir, maxw=1):
    """Walrus here rejects >1 sync wait per instruction; hoist extras onto
    preceding same-engine NoOps."""
    n = 0
    for f in nc.m.functions:
        for blk in f.blocks:
            insts = blk.instructions
            i = 0
            while i < len(insts):
                inst = insts[i]
                si = inst.sync_info
                if si is not None and len(si.on_wait) > maxw:
                    waits = list(si.on_wait)
                    si.on_wait = waits[:maxw]
                    rest = waits[maxw:]
                    for j in range(0, len(rest), maxw):
                        n += 1
                        nop = mybir.InstNoOp(
                            name=f"I-wsplit-{n}-{inst.name}",
                            engine=inst.engine,
                            ins=[],
                            outs=[],
                            sync_info=mybir.SyncInfo(
                                on_wait=list(rest[j : j + maxw]), on_update=[]
                            ),
                        )
                        nc.register_instruction(nop)
                        insts.insert(i, nop)
                        i += 1
                i += 1
    return n


def _build_nc(loop_n=1, bcast_mode="dma", min_engine="vector", bbufs=16):
    import contextlib
    import concourse.bass as bass
    import concourse.tile as tile
    import concourse.mybir as mybir
    from concourse.vector_clock import ScopedClock

    _patch_tile_drain(tile, mybir, ScopedClock)

    DT = mybir.dt.float32
    nc = bass.Bass("TRN2", debug=False)
    xs = nc.dram_tensor("xs", [MC, K], DT, kind="ExternalInput")
    wt = nc.dram_tensor("wt", [K, N], DT, kind="ExternalInput")  # weight.T
    bias = nc.dram_tensor("bias_in", [N], DT, kind="ExternalInput")
    out = nc.dram_tensor("out", [MC, N], DT, kind="ExternalOutput")
    with tile.TileContext(nc) as tc:
        loop_cm = tc.For_i(0, loop_n, 1) if loop_n > 1 else contextlib.nullcontext()
        with loop_cm, tc.tile_pool(name="singles", bufs=1) as singles, tc.tile_pool(
            name="bpool", bufs=bbufs
        ) as bpool:
            x_re = xs.ap().rearrange("(t p) k -> t p k", p=128)
            o_re = out.ap().rearrange("(t p) n -> t p n", p=128)
            ntiles = MC // 128
            x_sb, amax, amin = [], [], []
            for t in range(ntiles):
                xt = singles.tile([128, K], DT, tag=f"x{t}")
                nc.sync.dma_start(out=xt[:], in_=x_re[t])
                x_sb.append(xt)
                mx = singles.tile([128, N], DT, tag=f"amax{t}")
                mn = singles.tile([128, N], DT, tag=f"amin{t}")
                nc.vector.memset(mx[:], -FMAX)
                nc.vector.memset(mn[:], FMAX)
                amax.append(mx)
                amin.append(mn)
            biasb = singles.tile([128, N], DT, tag="biasb")
            bap = bias.ap()
            nc.sync.dma_start(
                out=biasb[:],
                in_=bass.AP(
                    tensor=bap.tensor, offset=bap.offset, ap=[[0, 128], list(bap.ap[0])]
                ),
            )
            if bcast_mode == "none":
                bt_static = singles.tile([128, N], DT, tag="bstatic")
                nc.vector.memset(bt_static[:], 0.01)
            for k in range(K):
                if bcast_mode == "none":
                    bt = bt_static
                else:
                    bt = bpool.tile([128, N], DT, tag="b")
                    row = wt.ap()[k : k + 1, :]
                    nc.sync.dma_start(
                        out=bt[:],
                        in_=bass.AP(
                            tensor=row.tensor,
                            offset=row.offset,
                            ap=[[0, 128], list(row.ap[1])],
                        ),
                    )
                for t in range(ntiles):
                    nc.vector.scalar_tensor_tensor(
                        out=amax[t][:],
                        in0=bt[:],
                        scalar=x_sb[t][:, k : k + 1],
                        in1=amax[t][:],
                        op0=mybir.AluOpType.mult,
                        op1=mybir.AluOpType.max,
                    )
                    min_eng = nc.gpsimd if min_engine == "gpsimd" else nc.vector
                    min_eng.scalar_tensor_tensor(
                        out=amin[t][:],
                        in0=bt[:],
                        scalar=x_sb[t][:, k : k + 1],
                        in1=amin[t][:],
                        op0=mybir.AluOpType.mult,
                        op1=mybir.AluOpType.min,
                    )
            for t in range(ntiles):
                nc.vector.tensor_tensor(
                    out=amax[t][:],
                    in0=amax[t][:],
                    in1=amin[t][:],
                    op=mybir.AluOpType.add,
                )
                nc.vector.tensor_tensor(
                    out=amax[t][:],
                    in0=amax[t][:],
                    in1=biasb[:],
                    op=mybir.AluOpType.add,
                )
                nc.sync.dma_start(out=o_re[t], in_=amax[t][:])
    _split_sem_waits(nc, mybir)
    return nc


def _build_nc_v2(loop_n=1):
    """Scan-based implementation.

    Per core: output C^T [N, MC] (host transposes back).
      - PE materializes product tiles p[j, k] = W[j,k]*x[i,k] for each row i
        and jb block, via 6 chunk matmuls lhsT=WT[kc,jb] @ rhs=diag(x[i,kc]).
      - diag(x[i,kc]) built with one tensor_scalar (identity * per-partition x^T col).
      - ScalarE copies the second k-half of p from PSUM to SBUF.
      - DVE tensor_tensor_scan(max/min) consumes TWO product streams per cycle
        (PSUM half + SBUF half); the running state is written with a
        zero-stride AP so the final value lands in C^T[jb][:, i].
    """
    import contextlib
    import concourse.bass as bass
    import concourse.tile as tile
    import concourse.mybir as mybir
    from concourse.vector_clock import ScopedClock

    _patch_tile_drain(tile, mybir, ScopedClock)

    DT = mybir.dt.float32
    JB = N // 128
    KC = K // 128
    HALF = K // 2
    nc = bass.Bass("TRN2", debug=False)
    xt = nc.dram_tensor("xt", [K, MC], DT, kind="ExternalInput")  # x^T shard
    wt = nc.dram_tensor("wt", [K, N], DT, kind="ExternalInput")  # weight.T
    bias = nc.dram_tensor("bias_in", [N], DT, kind="ExternalInput")
    ident = nc.dram_tensor("ident", [128, 128], DT, kind="ExternalInput")
    ct = nc.dram_tensor("ct", [N, MC], DT, kind="ExternalOutput")  # C^T
    mx = mybir.AluOpType.max
    mn = mybir.AluOpType.min
    with tile.TileContext(nc) as tc:
        loop_cm = tc.For_i(0, loop_n, 1) if loop_n > 1 else contextlib.nullcontext()
        with loop_cm, tc.tile_pool(name="singles", bufs=1) as singles, tc.tile_pool(
            name="dgpool", bufs=2
        ) as dgpool, tc.tile_pool(name="pspool", bufs=3, space="PSUM") as pspool, tc.tile_pool(
            name="sbpool", bufs=4
        ) as sbpool:
            id_sb = singles.tile([128, 128], DT, tag="ident")
            nc.sync.dma_start(out=id_sb[:], in_=ident.ap())
            # WT chunk tiles [kc][jb] : [128, 128]
            w_sb = [
                [singles.tile([128, 128], DT, tag=f"w{c}_{b}", name=f"w{c}_{b}") for b in range(JB)]
                for c in range(KC)
            ]
            for c in range(KC):
                for b in range(JB):
                    nc.sync.dma_start(
                        out=w_sb[c][b][:],
                        in_=wt.ap()[c * 128 : (c + 1) * 128, b * 128 : (b + 1) * 128],
                    )
            xt_sb = [singles.tile([128, MC], DT, tag=f"xt{c}", name=f"xt{c}") for c in range(KC)]
            for c in range(KC):
                nc.sync.dma_start(
                    out=xt_sb[c][:], in_=xt.ap()[c * 128 : (c + 1) * 128, :]
                )
            bias_sb = singles.tile([128, JB], DT, tag="bias")
            nc.sync.dma_start(
                out=bias_sb[:], in_=bias.ap().rearrange("(b p) -> p b", p=128)
            )
            ctmax = [singles.tile([128, MC], DT, tag=f"ctmax{b}", name=f"ctmax{b}") for b in range(JB)]
            ctmin = [singles.tile([128, MC], DT, tag=f"ctmin{b}", name=f"ctmin{b}") for b in range(JB)]
            for i in range(MC):
                dgs = []
                for c in range(KC):
                    dg = dgpool.tile([128, 128], DT, tag=f"dg{c}")
                    nc.vector.tensor_scalar(
                        out=dg[:],
                        in0=id_sb[:],
                        scalar1=xt_sb[c][:, i : i + 1],
                        scalar2=None,
                        op0=mybir.AluOpType.mult,
                    )
                    dgs.append(dg)
                for b in range(JB):
                    p_ps = pspool.tile([128, K], DT, tag="pp")
                    for c in range(KC):
                        nc.tensor.matmul(
                            out=p_ps[:, c * 128 : (c + 1) * 128],
                            lhsT=w_sb[c][b][:],
                            rhs=dgs[c][:],
                            start=True,
                            stop=True,
                        )
                    p_sb = sbpool.tile([128, HALF], DT, tag="psb")
                    nc.scalar.copy(out=p_sb[:], in_=p_ps[:, HALF:])
                    for acc_t, op, init in (
                        (ctmax[b], mx, -FMAX),
                        (ctmin[b], mn, FMAX),
                    ):
                        base = acc_t[:, i : i + 1]
                        outap = bass.AP(
                            tensor=base.tensor,
                            offset=base.offset,
                            ap=[list(base.ap[0]), [0, HALF]],
                        )
                        nc.vector.tensor_tensor_scan(
                            out=outap,
                            data0=p_ps[:, :HALF],
                            data1=p_sb[:],
                            initial=init,
                            op0=op,
                            op1=op,
                        )
            ct_re = ct.ap().rearrange("(b p) m -> b p m", p=128)
            for b in range(JB):
                nc.vector.tensor_tensor(
                    out=ctmax[b][:], in0=ctmax[b][:], in1=ctmin[b][:],
                    op=mybir.AluOpType.add,
                )
                nc.vector.tensor_scalar(
                    out=ctmax[b][:], in0=ctmax[b][:],
                    scalar1=bias_sb[:, b : b + 1], scalar2=None,
                    op0=mybir.AluOpType.add,
                )
                nc.sync.dma_start(out=ct_re[b], in_=ctmax[b][:])
    _split_sem_waits(nc, mybir)
    return nc


def _make_runner(nc, n_cores=NCORES):
    import jax
    from jax.sharding import Mesh, PartitionSpec
    from jax.experimental.shard_map import shard_map
    import concourse.mybir as mybir
    from concourse import bass2jax

    bass2jax.install_neuronx_cc_hook()

    partition_name = (
        nc.partition_id_tensor.name if nc.partition_id_tensor else None
    )
    in_names, out_names, out_avals, zero_shapes = [], [], [], []
    for alloc in nc.m.functions[0].allocations:
        if not isinstance(alloc, mybir.MemoryLocationSet):
            continue
        name = alloc.memorylocations[0].name
        if alloc.kind == "ExternalInput":
            if name != partition_name:
                in_names.append(name)
        elif alloc.kind == "ExternalOutput":
            shape = tuple(alloc.tensor_shape)
            dtype = mybir.dt.np(alloc.dtype)
            out_names.append(name)
            out_avals.append(jax.core.ShapedArray(shape, dtype))
            zero_shapes.append((shape, dtype))
    n_params = len(in_names)
    n_outs = len(out_avals)
    in_names_all = list(in_names) + list(out_names)
    if partition_name is not None:
        in_names_all.append(partition_name)

    def _body(*args):
        operands = list(args)
        if partition_name is not None:
            operands.append(bass2jax.partition_id_tensor())
        outs = bass2jax._bass_exec_p.bind(
            *operands,
            out_avals=tuple(out_avals),
            in_names=tuple(in_names_all),
            out_names=tuple(out_names),
            lowering_input_output_aliases=(),
            sim_require_finite=True,
            sim_require_nnan=True,
            nc=nc,
        )
        return tuple(outs)

    devices = jax.devices()[:n_cores]
    mesh = Mesh(np.asarray(devices), ("core",))
    in_specs = (PartitionSpec("core"),) * (n_params + n_outs)
    out_specs = (PartitionSpec("core"),) * n_outs
    sharded = jax.jit(
        shard_map(
            _body, mesh=mesh, in_specs=in_specs, out_specs=out_specs, check_rep=False
        ),
        keep_unused=True,
    )

    def run(in_maps):
        global LAST_RUN_SECONDS
        import time as _time

        per_core = [[np.asarray(m[nm]) for nm in in_names] for m in in_maps]
        concat_in = [
            np.concatenate([per_core[c][i] for c in range(n_cores)], axis=0)
            for i in range(n_params)
        ]
        concat_zeros = [
            np.zeros((n_cores * s[0], *s[1:]), d) for (s, d) in zero_shapes
        ]
        t0 = _time.time()
        out_arrs = sharded(*concat_in, *concat_zeros)
        out_np = [np.asarray(a) for a in out_arrs]
        LAST_RUN_SECONDS = _time.time() - t0
        return [
            {
                nm: out_np[i].reshape(n_cores, *out_avals[i].shape)[c]
                for i, nm in enumerate(out_names)
            }
            for c in range(n_cores)
        ]

    run.sharded = sharded
    run.in_names = in_names
    run.zero_shapes = zero_shapes
    run.out_names = out_names
    run.out_avals = out_avals
    run.mesh = mesh
    return run


IMPL = os.environ.get("MAM_IMPL", "v1")


def _fallback_runner(nc):
    from concourse.bass_utils import run_bass_kernel_spmd

    def run(in_maps):
        res = run_bass_kernel_spmd(nc, in_maps, core_ids=list(range(NCORES)))
        return res.results

    return run


def _get_runner():
    if "runner" not in _STATE:
        impl = IMPL
        try:
            nc = _build_nc_v2() if impl == "v2" else _build_nc()
            _STATE["runner"] = _make_runner(nc)
            _STATE["impl"] = impl
        except Exception:
            nc = _build_nc()
            _STATE["runner"] = _fallback_runner(nc)
            _STATE["impl"] = "v1"
    return _STATE["runner"], _STATE["impl"]


def _run_with_retry(run, in_maps):
    try:
        return run(in_maps)
    except Exception:
        # Transient device errors have been observed; retry once through the
        # slower standard execution path on a freshly built program.
        _STATE.pop("runner", None)
        nc = _build_nc_v2() if _STATE.get("impl") == "v2" else _build_nc()
        run2 = _fallback_runner(nc)
        return run2(in_maps)


def kernel(x, weight, bias):
    x = np.ascontiguousarray(np.asarray(x, dtype=np.float32))
    W = np.ascontiguousarray(np.asarray(weight, dtype=np.float32))
    b = np.ascontiguousarray(np.asarray(bias, dtype=np.float32))
    run, impl = _get_runner()
    xf = x.reshape(-1, K)
    wt = np.ascontiguousarray(W.T)
    if impl == "v2":
        ident = np.eye(128, dtype=np.float32)
        in_maps = [
            {
                "xt": np.ascontiguousarray(xf[c * MC : (c + 1) * MC].T),
                "wt": wt,
                "bias_in": b,
                "ident": ident,
            }
            for c in range(NCORES)
        ]
        outs = _run_with_retry(run, in_maps)
        C = np.concatenate([o["ct"].T for o in outs], axis=0)
    else:
        in_maps = [
            {"xs": xf[c * MC : (c + 1) * MC], "wt": wt, "bias_in": b}
            for c in range(NCORES)
        ]
        outs = _run_with_retry(run, in_maps)
        C = np.concatenate([o["out"] for o in outs], axis=0)
    return np.ascontiguousarray(
        C.reshape(x.shape[:-1] + (W.shape[0],)), dtype=np.float32
    )
